# revision 14
# baseline (speedup 1.0000x reference)
"""Trainium2 Bass kernel for batched multi-head causal attention.

Sharding: 8 cores = 4 batches x 2 head-halves; host sums the two partial
outputs per batch. No collectives.

One-pass flash-style attention in [q,k] layout:
per 128-query tile, S is computed into PSUM (hi/lo fp16, see below), the
causal diag block is masked by a DVE add, DVE reduces the exact row max,
and ACT applies exp with per-partition bias = -max straight out of PSUM.
The softmax normalizer l is accumulated exactly in fp32 via a fused ones
column in AV, and O is divided by l at the end, so the shift is exact.

Precision: logits need ~16+ mantissa bits (near-tie softmax rows amplify
logit noise). q/k chains run as fp16 hi+lo pairs. Projections use the
classic 3 products (hi*hi + lo*hi + hi*lo). The S matmul exploits that PE
cost depends only on the moving free size, not the contraction dim: the
hi*hi and lo*hi products are stacked into ONE K=128 matmul with
lhsT=[qhi;qlo] against rhs=[khi;khi], plus a K=64 matmul for qhi*klo --
2 instructions instead of 3.

All transposes run on the DMA xbar engines (idle during attention), not
the PE: P panels are block-transposed SBUF->SBUF with a single
dma_start_transpose whose 3D out AP folds rows into 128-key chunks
(PT[p, b, q] = P[q, 128b+p]); the per-pair output panel transposes the
same way into oT. The PE therefore runs only S and AV matmuls in phase A.

The two heads of a pair are SKEWED: head A ascends q-tiles 0..15 while
head B descends 15..0, so every iteration has the same S/exp/AV element
count and the in-flight S PSUM demand is a constant 5 banks (pool of 6).

Per-core pipeline:
  P. DMA loads (hi/lo fp16, host-pretransposed) + projections; build
     q_stack=[qhi;qlo], ksA=[khi;khi], klo pair tiles, vha=[vh|1].
  A. Per head pair, skewed per-tile loop: S chunks (2 matmuls each) ->
     mask -> DVE max -> ACT exp(bias=-m) -> fp16 P -> xbar transpose to
     PT; one tile behind: AV matmuls (lhsT=PT chunk, rhs=vha) accumulate
     [O|l] in PSUM; recip(l), ACT scale into the onp panel; at pair end
     one xbar transpose onp -> oT.
  O. Output projection (fp16) -> ACT copy -> DMA out.
"""

import os
import sys
import types
from contextlib import ExitStack

import numpy as np

if "/opt/trn_rl_repo" not in sys.path:
    sys.path.insert(0, "/opt/trn_rl_repo")

try:  # NTFF profile hook shim (image's antenv lacks axon_hooks)
    import antenv.axon_hooks  # noqa: F401
except ImportError:
    try:
        from trn_agent_boot.trn_boot import _ntff_profile_via_ctypes

        _mod = types.ModuleType("antenv.axon_hooks")
        _hook = _ntff_profile_via_ctypes("/opt/axon/libaxon_pjrt.so")
        _mod.get_axon_ntff_profile_hook = lambda: _hook
        sys.modules["antenv.axon_hooks"] = _mod
    except Exception:
        pass

import concourse.bass as bass
import concourse.mybir as mybir
import concourse.tile as tile

f32 = mybir.dt.float32
f16 = mybir.dt.float16
AF = mybir.ActivationFunctionType
ALU = mybir.AluOpType
AX = mybir.AxisListType

N_CORES = 8
B, N, D = 4, 2048, 1024
H, DK, DV = 16, 64, 64
HL = 8  # heads per core
NEG = -1e9

TRACE = False
LDW_STRIP = True
LAST_RESULT = {}


def build_nc(do_split: bool = True):
    nc = bass.Bass()
    dp = nc.declare_dram_parameter
    qhi_d = dp("qthi", [D, N], f16, isOutput=False)
    qlo_d = dp("qtlo", [D, N], f16, isOutput=False)
    khi_d = dp("kthi", [D, N], f16, isOutput=False)
    klo_d = dp("ktlo", [D, N], f16, isOutput=False)
    vt_d = dp("vt", [D, N], f16, isOutput=False)
    wqhi_d = dp("wqhi", [D, HL * DK], f16, isOutput=False)
    wqlo_d = dp("wqlo", [D, HL * DK], f16, isOutput=False)
    wkhi_d = dp("wkhi", [D, HL * DK], f16, isOutput=False)
    wklo_d = dp("wklo", [D, HL * DK], f16, isOutput=False)
    wv_d = dp("wv", [D, HL * DV], f16, isOutput=False)
    wo_d = dp("wo", [HL * DV, D], f16, isOutput=False)
    out_d = dp("out", [N, D], f16, isOutput=True)

    with tile.TileContext(nc) as tc, ExitStack() as ctx:
        build_body(nc, tc, ctx, (qhi_d, qlo_d), (khi_d, klo_d), vt_d,
                   (wqhi_d, wqlo_d), (wkhi_d, wklo_d), wv_d, wo_d, out_d)

    if LDW_STRIP:
        strip_redundant_ldweights(nc)
    if do_split:
        split_excess_waits(nc)
    return nc


def strip_redundant_ldweights(nc):
    """Set ldweights=False on matmuls whose stationary operand is identical
    to the immediately preceding PE matmul's (no intervening PE work), so
    codegen skips the redundant weight reload."""
    n = 0
    for f in nc.m.functions:
        for bb in f.blocks:
            prev_key = None
            for inst in bb.instructions:
                if inst.engine != mybir.EngineType.PE:
                    continue
                if isinstance(inst, mybir.InstMatmult) and not inst.is_transpose:
                    key = (repr(inst.ins[1]), inst.perf_mode,
                           inst.tile_position)
                    if key == prev_key:
                        inst.ldweights = False
                        n += 1
                    prev_key = key
                else:
                    prev_key = None
    return n


def build_body(nc, tc, ctx, q_ds, k_ds, vt_d, wq_ds, wk_ds, wv_d, wo_d, out_d):
    # ---------------- constants ----------------
    consts = ctx.enter_context(tc.tile_pool(name="consts", bufs=1))
    # causal mask for the diag 128x128 block: NEG where key(free) > q(part)
    maskC = consts.tile([128, 128], f32)
    nc.gpsimd.memset(maskC, 0.0)
    nc.gpsimd.affine_select(
        out=maskC, in_=maskC, compare_op=ALU.is_ge, fill=NEG,
        base=0, pattern=[[-1, 128]], channel_multiplier=1,
    )

    # ---------------- persistent tiles ----------------
    projp = ctx.enter_context(tc.tile_pool(name="proj", bufs=1))
    # per head: [qhi;qlo] (A-heads) / [qlo;qhi] (B-heads) on partitions
    qs = [projp.tile([128, N], f16, name=f"qs{h}") for h in range(HL)]
    # per head: [khi;khi] duplicated on both partition halves
    ksA = [projp.tile([128, N], f16, name=f"ksA{h}") for h in range(HL)]
    # pair layout: head 2m at partitions 0-63, head 2m+1 at 64-127
    kloT = [projp.tile([128, N], f16, name=f"kloT{m}") for m in range(4)]
    # vh_aug: per head, 16 chunks of [128, 65]; col 64 stays 1.0 (memset)
    vha = [projp.tile([128, 16 * 65], f16, name=f"vha{h}") for h in range(HL)]
    for h in range(HL):
        nc.vector.memset(vha[h], 1.0)
    oT = [projp.tile([128, N], f16, name=f"oT{m}") for m in range(4)]

    # ---------------- phase P: loads + projections ----------------
    dmae = [nc.sync, nc.scalar]  # two HW DGE queues
    with tc.tile_pool(name="wx", bufs=1) as wpool, \
         tc.tile_pool(name="xt", bufs=2) as xt_pool, \
         tc.tile_pool(name="hi16", bufs=3) as hi_pool, \
         tc.tile_pool(name="lo16", bufs=3) as lo_pool, \
         tc.tile_pool(name="pp", bufs=3, space="PSUM") as pp:
        for kind in ("k", "v", "q"):
            if kind == "v":
                srcs = [(vt_d, wv_d)]
            elif kind == "q":
                srcs = [(q_ds[0], wq_ds[0]), (q_ds[1], wq_ds[1])]
            else:
                srcs = [(k_ds[0], wk_ds[0]), (k_ds[1], wk_ds[1])]
            w_sb = []
            for si, (xd, wd) in enumerate(srcs):
                w = wpool.tile([128, 8 * 512], f16, tag=f"w{si}", name=f"w{kind}{si}")
                w_sb.append(w)
                for d in range(8):
                    dmae[d % 2].dma_start(
                        out=w[:, d * 512:(d + 1) * 512],
                        in_=wd[d * 128:(d + 1) * 128, :],
                    )
            for j in range(4):  # 512-col blocks of the sequence
                xts = []
                for si, (xd, wd) in enumerate(srcs):
                    xt = xt_pool.tile([128, 8 * 512], f16, tag=f"xt{si}",
                                      name=f"xt{kind}{si}{j}")
                    xts.append(xt)
                    for d in range(8):
                        dmae[(d + si) % 2].dma_start(
                            out=xt[:, d * 512:(d + 1) * 512],
                            in_=xd[d * 128:(d + 1) * 128, j * 512:(j + 1) * 512],
                        )
                for m in range(4):  # head-pair groups / v: seq sub-chunks
                    ps = pp.tile([128, 512], f32, tag="pp")
                    if kind == "v":
                        # flipped: lhsT = v-chunk (seq on free), rhs = wv;
                        # out = vh [128 seq, 512 head-dims] -- natural
                        # key-major layout, no transposes needed.
                        for d in range(8):
                            nc.tensor.matmul(
                                ps,
                                lhsT=xts[0][:, d * 512 + m * 128:
                                            d * 512 + m * 128 + 128],
                                rhs=w_sb[0][:, d * 512:(d + 1) * 512],
                                start=(d == 0), stop=(d == 7),
                            )
                        kc = j * 4 + m
                        for h in range(HL):
                            # vh scaled by 1/64 so the unnormalized O^T
                            # panel fits fp16; undone by linv = 64/l.
                            nc.vector.tensor_scalar_mul(
                                vha[h][:, kc * 65:kc * 65 + 64],
                                ps[:, h * 64:h * 64 + 64],
                                1.0 / 64)
                        continue
                    # q/k: products hi*hi, hi*lo, lo*hi (lo*lo dropped)
                    prods = [(w_sb[0], xts[0]), (w_sb[0], xts[1]),
                             (w_sb[1], xts[0])]
                    first = True
                    for pi, (wt, xt) in enumerate(prods):
                        for d in range(8):
                            nc.tensor.matmul(
                                ps,
                                lhsT=wt[:, d * 512 + m * 128:d * 512 + m * 128 + 128],
                                rhs=xt[:, d * 512:(d + 1) * 512],
                                start=first,
                                stop=(pi == len(prods) - 1 and d == 7),
                            )
                            first = False
                    js = slice(j * 512, (j + 1) * 512)
                    hA, hB = 2 * m, 2 * m + 1
                    if kind == "q":
                        hi = hi_pool.tile([128, 512], f16, tag="hi")
                        nc.scalar.copy(hi, ps)
                        lo = lo_pool.tile([128, 512], f16, tag="lo")
                        nc.vector.tensor_tensor(lo, ps, hi, ALU.subtract)
                        nc.gpsimd.tensor_copy(qs[hA][0:64, js], hi[0:64, :])
                        nc.gpsimd.tensor_copy(qs[hB][64:128, js], hi[64:128, :])
                        # partition-shifted SBUF->SBUF moves for the lo halves
                        dmae[m % 2].dma_start(out=qs[hA][64:128, js],
                                              in_=lo[0:64, :])
                        dmae[(m + 1) % 2].dma_start(out=qs[hB][0:64, js],
                                                    in_=lo[64:128, :])
                    else:  # k
                        hi = hi_pool.tile([128, 512], f16, tag="hi")
                        nc.scalar.copy(hi, ps)
                        nc.vector.tensor_tensor(kloT[m][:, js], ps, hi,
                                                ALU.subtract)
                        nc.gpsimd.tensor_copy(ksA[hA][0:64, js], hi[0:64, :])
                        nc.gpsimd.tensor_copy(ksA[hB][64:128, js], hi[64:128, :])
                        dmae[m % 2].dma_start(out=ksA[hA][64:128, js],
                                              in_=hi[0:64, :])
                        dmae[(m + 1) % 2].dma_start(out=ksA[hB][0:64, js],
                                                    in_=hi[64:128, :])

    # ---------------- phase A: attention ----------------
    # Head A ascends q-tiles 0..15 while head B descends 15..0 (constant
    # per-iteration load, <=5 S banks in flight). All transposes go via
    # the DMA xbar; PE does only S and AV matmuls.
    with tc.tile_pool(name="sp", bufs=6, space="PSUM") as sp, \
         tc.tile_pool(name="oacc", bufs=2, space="PSUM") as oaccp, \
         tc.tile_pool(name="pb", bufs=4) as pbuf, \
         tc.tile_pool(name="ptb", bufs=6) as ptb, \
         tc.tile_pool(name="stg", bufs=3) as stgp, \
         tc.tile_pool(name="lp", bufs=1) as lpp, \
         tc.tile_pool(name="lv", bufs=2) as lvp, \
         tc.tile_pool(name="stats", bufs=24) as stats:

        def s_block(h, t, dq):
            mg, a = h // 2, h % 2
            hi_sl = slice(0, 64) if a == 0 else slice(64, 128)
            klo_sl = slice(a * 64, (a + 1) * 64)
            E = 128 * (t + 1)
            nch = (E + 511) // 512
            P = pbuf.tile([128, N], f16, tag="p", name=f"P{h}_{t}")
            schunks = []
            for c in range(nch):
                w = min(512, E - 512 * c)
                sc = sp.tile([128, 512], f32, tag="s", name=f"S{h}_{t}_{c}")
                schunks.append((sc, w))
            # all instA chunks first, then all instB: adjacent matmuls
            # share lhsT so the ldweights post-pass can strip reloads
            for c, (sc, w) in enumerate(schunks):
                nc.tensor.matmul(
                    sc[:, 0:w],
                    lhsT=qs[h][:, t * 128:(t + 1) * 128],
                    rhs=ksA[h][:, c * 512:c * 512 + w],
                    start=True, stop=False,
                )
            for c, (sc, w) in enumerate(schunks):
                nc.tensor.matmul(
                    sc[:, 0:w],
                    lhsT=qs[h][hi_sl, t * 128:(t + 1) * 128],
                    rhs=kloT[mg][klo_sl, c * 512:c * 512 + w],
                    start=False, stop=True,
                )
            lsc, lw = schunks[-1]
            nc.vector.tensor_tensor(
                lsc[:, lw - 128:lw], lsc[:, lw - 128:lw], maskC, ALU.add)
            negM = stats.tile([128, 1], f32, tag="negM", name=f"negM{h}_{t}")
            for c, (sc, w) in enumerate(schunks):
                if c == 0:
                    nc.vector.tensor_reduce(negM, sc[:, 0:w], AX.X,
                                            ALU.max, negate=True)
                else:
                    nm2 = stats.tile([128, 1], f32, tag="nm2",
                                     name=f"nm2_{h}_{t}_{c}")
                    nc.vector.tensor_reduce(nm2, sc[:, 0:w], AX.X,
                                            ALU.max, negate=True)
                    nc.vector.tensor_tensor(negM, negM, nm2, ALU.min)
            for c, (sc, w) in enumerate(schunks):
                nc.scalar.activation(P[:, c * 512:c * 512 + w],
                                     sc[:, 0:w], AF.Exp, bias=negM)
            # blocked transpose of the P panel on the DMA xbar:
            # PT[p, b, q] = P[q, 128*b + p]. All xbar transposes go through
            # ONE queue (serialized) and are consumed two iterations later.
            PT = ptb.tile([128, N], f16, tag="pt", name=f"PT{h}_{t}")
            dq.dma_start_transpose(
                PT[:, 0:E].rearrange("p (b q) -> p b q", q=128),
                P[:, 0:E])
            return PT

        def av_block(h, tprev, PT_, lpan):
            # flipped AV: lhsT = vha chunk (65-col weight load), rhs = PT
            # chunk (128-row stream) -> O^T [65, 128q] accumulates directly
            # in oT layout; row 64 is l (ones column). Stream-dominated so
            # the PE array stays busy and HAM keeps the clock up.
            mg, a = h // 2, h % 2
            nkc = tprev + 1
            o = oaccp.tile([128, 512], f32, tag="o", name=f"O{h}_{tprev}")
            for kc in range(nkc):
                nc.tensor.matmul(
                    o[0:65, 0:128],
                    lhsT=vha[h][:, kc * 65:(kc + 1) * 65],
                    rhs=PT_[:, kc * 128:(kc + 1) * 128],
                    start=(kc == 0), stop=(kc == nkc - 1),
                )
            cs = slice(tprev * 128, (tprev + 1) * 128)
            nc.scalar.activation(lpan[64:65, (a * N + tprev * 128):
                                       (a * N + (tprev + 1) * 128)],
                                 o[64:65, 0:128], AF.Copy, scale=1.0 / 64)
            if a == 0:
                nc.scalar.activation(oT[mg][0:64, cs], o[0:64, 0:128],
                                     AF.Copy)
            else:
                # dv rows belong at oT partitions 64-127: ACT can't cross
                # lanes, so stage in SBUF and partition-shift via DMA.
                stg = stgp.tile([64, 128], f16, tag="stg",
                                name=f"stg{h}_{tprev}")
                nc.scalar.activation(stg, o[0:64, 0:128], AF.Copy)
                dmae[0].dma_start(out=oT[mg][64:128, cs], in_=stg)

        for mg in range(4):
            hA, hB = 2 * mg, 2 * mg + 1
            # l rows for the pair: head A at cols 0:N, head B at N:2N,
            # both on partition 64 (the ones-column row of O^T).
            lpan = lpp.tile([65, 2 * N], f16, tag="lp", name=f"lpan{mg}")
            ptA = ptB = ptA2 = ptB2 = None
            for i in range(18):
                tA, tB = i, 15 - i
                newA = s_block(hA, tA, dmae[0]) if i < 16 else None
                if i >= 2:
                    av_block(hB, 17 - i, ptB2, lpan)
                newB = s_block(hB, tB, dmae[0]) if i < 16 else None
                if i >= 2:
                    av_block(hA, i - 2, ptA2, lpan)
                ptA2, ptB2 = ptA, ptB
                ptA, ptB = newA, newB
            # normalize the pair panel: broadcast l/64 down the partition
            # dim via stride-0 DMA reads, DVE reciprocal -> 64/l, then one
            # DVE multiply.
            linv = lvp.tile([128, N], f16, tag="lv", name=f"linv{mg}")
            dmae[0].dma_start(
                out=linv[0:64, :],
                in_=lpan[64:65, 0:N].unsqueeze(1).broadcast_to([1, 64, N]))
            dmae[0].dma_start(
                out=linv[64:128, :],
                in_=lpan[64:65, N:2 * N].unsqueeze(1).broadcast_to(
                    [1, 64, N]))
            with nc.allow_low_precision(reason="1/l in fp16: 5e-4 rel, "
                                        "well inside the 2e-2 gate"):
                nc.vector.reciprocal(linv, linv)
            nc.vector.tensor_tensor(oT[mg], oT[mg], linv, ALU.mult)

    # ---------------- phase O: output projection ----------------
    with tc.tile_pool(name="wo", bufs=1) as wopool, \
         tc.tile_pool(name="osb2", bufs=4) as osbpool, \
         tc.tile_pool(name="op", bufs=4, space="PSUM") as oppool:
        wo_sb = wopool.tile([128, 4 * 1024], f16)
        for hc in range(4):
            nc.sync.dma_start(
                out=wo_sb[:, hc * 1024:(hc + 1) * 1024],
                in_=wo_d[hc * 128:(hc + 1) * 128, :],
            )
        for rt in range(16):
            pss = [oppool.tile([128, 512], f32, tag="op", name=f"op{rt}_{dh}")
                   for dh in range(2)]
            # hc outer, dh inner: adjacent matmuls share lhsT
            for hc in range(4):
                for dh in range(2):
                    nc.tensor.matmul(
                        pss[dh],
                        lhsT=oT[hc][:, rt * 128:(rt + 1) * 128],
                        rhs=wo_sb[:, hc * 1024 + dh * 512:hc * 1024 + dh * 512 + 512],
                        start=(hc == 0), stop=(hc == 3),
                    )
            for dh in range(2):
                ost = osbpool.tile([128, 512], f16, tag="osb")
                nc.scalar.copy(ost, pss[dh])
                dmae[dh].dma_start(
                    out=out_d[rt * 128:(rt + 1) * 128, dh * 512:(dh + 1) * 512],
                    in_=ost,
                )


def split_excess_waits(nc, maxw: int = 1):
    """Hoist excess per-instruction sync waits onto preceding NOPs (this
    walrus's CTRL encoding takes only one wait)."""
    n_split = 0
    for f in nc.m.functions:
        for bb in f.blocks:
            insts = bb.instructions
            i = 0
            while i < len(insts):
                inst = insts[i]
                si = inst.sync_info
                if si is not None and len(si.on_wait) > maxw:
                    waits = list(si.on_wait)
                    keep = waits[-maxw:]
                    excess = waits[:-maxw]
                    pos = i
                    for j in range(0, len(excess), maxw):
                        chunk = excess[j:j + maxw]
                        nop = mybir.InstNoOp(
                            name=f"{inst.name}-wsplit{j}",
                            engine=inst.engine,
                            sync_info=mybir.SyncInfo(on_wait=chunk, on_update=[]),
                            bass_nofuse=True,
                        )
                        insts.insert(pos, nop)
                        pos += 1
                        i += 1
                    inst.sync_info = mybir.SyncInfo(
                        on_wait=keep, on_update=list(si.on_update)
                    )
                    n_split += 1
                i += 1
    return n_split


_NC_CACHE = {}


def get_nc():
    if "nc" not in _NC_CACHE:
        _NC_CACHE["nc"] = build_nc()
    return _NC_CACHE["nc"]


def _split16(x):
    hi = x.astype(np.float16)
    lo = (x - hi.astype(np.float32)).astype(np.float16)
    return np.ascontiguousarray(hi), np.ascontiguousarray(lo)


def make_in_maps(q, k, v, w_q, w_k, w_v, w_o):
    q = np.asarray(q, dtype=np.float32)
    k = np.asarray(k, dtype=np.float32)
    v = np.asarray(v, dtype=np.float32)
    w_q = np.asarray(w_q, dtype=np.float32)
    w_k = np.asarray(w_k, dtype=np.float32)
    w_v = np.asarray(w_v, dtype=np.float32)
    w_o = np.asarray(w_o, dtype=np.float32)
    scale = np.float32(1.0 / np.sqrt(np.float32(DK)))
    in_maps = []
    for c in range(N_CORES):
        b, half = c // 2, c % 2
        hs = slice(half * HL, (half + 1) * HL)
        qhi, qlo = _split16(q[b].T)
        khi, klo = _split16(k[b].T)
        wq = w_q[hs].transpose(1, 0, 2).reshape(D, HL * DK) * scale
        wqhi, wqlo = _split16(wq)
        wk = w_k[hs].transpose(1, 0, 2).reshape(D, HL * DK)
        wkhi, wklo = _split16(wk)
        in_maps.append({
            "qthi": qhi, "qtlo": qlo, "kthi": khi, "ktlo": klo,
            "vt": np.ascontiguousarray(v[b].T.astype(np.float16)),
            "wqhi": wqhi, "wqlo": wqlo, "wkhi": wkhi, "wklo": wklo,
            "wv": np.ascontiguousarray(
                w_v[hs].transpose(1, 0, 2).reshape(D, HL * DV).astype(np.float16)),
            "wo": np.ascontiguousarray(
                w_o[half * HL * DV:(half + 1) * HL * DV].astype(np.float16)),
        })
    return in_maps


def kernel(q, k, v, w_q, w_k, w_v, w_o):
    from concourse.bass_utils import run_bass_kernel_spmd

    nc = get_nc()
    in_maps = make_in_maps(q, k, v, w_q, w_k, w_v, w_o)
    res = run_bass_kernel_spmd(nc, in_maps, list(range(N_CORES)), trace=TRACE)
    LAST_RESULT["exec_time_ns"] = res.exec_time_ns
    LAST_RESULT["mean_exec_time_ns"] = res.mean_exec_time_ns
    LAST_RESULT["res"] = res
    outs = [res.results[c]["out"].astype(np.float32) for c in range(N_CORES)]
    return np.stack([outs[2 * b] + outs[2 * b + 1] for b in range(B)])
